# revision 22
# baseline (speedup 1.0000x reference)
"""Distributed kNN retrieval kernel for Trainium2 (8 NeuronCores).

Computes, for query batch B=256 against three memory banks of N=131072 rows
(D=512): combined = (0.4*cos(q,Mq) + 0.4*cos(q,Mr) + 0.2*cos(q,Mt)) * strength,
masked below 0.3 to -1.0, then top-5 values + indices per query row
(ties broken by the lowest index, matching jax.lax.top_k).

The memory-side math is query-independent: cos(q, M_b) = q_hat . M_b_hat, so
  combined = q_hat @ E^T   with   E = sum_b w_b*strength/(||M_b||+eps) * M_b.
E is an index-time artifact (a real retrieval system stores normalized,
weighted embeddings); the host folds the three banks into E once, pre-packs
it in matmul (transposed) layout, and quantizes to bf16. The device does all
the query-dependent work.

The reference's 0.3 similarity threshold masks sub-threshold candidates to
-1.0, so the top-k output only ever contains above-threshold survivors (plus
deterministic -1/index fills). The kernel exploits this with the standard
threshold-pruned retrieval structure:

  Pass 1 (always): shard E along N across the 8 cores; each core normalizes
  the queries, streams its shard through the Tensor engine (q_hat @ E^T in
  [128, 512] PSUM tiles), and reduces each score tile to a per-(row, chunk)
  max on the DVE (tensor_reduce). The host gathers the tiny flag tensors and
  compares against the threshold.

  Pass 2 (only if some flag exceeds the threshold): rerun the shard with full
  top-8 extraction per 512-column chunk (DVE max/max_index, stable
  ascending-index tie-break), gather 8*32*8 candidates per row, and reduce to
  the global top-k on the host (value desc, index asc). Exactness: any
  element of the global top-5 has at most 4 elements above it anywhere, so it
  is inside its chunk's top-8.

  Rows with no survivors take the reference's tie-break on the -1.0 masked
  entries: value -1.0 with the smallest unoccupied indices, which the host
  emits directly.
"""

import sys

if "/opt/trn_rl_repo" not in sys.path:
    sys.path.insert(0, "/opt/trn_rl_repo")

import numpy as np

B = 256
D = 512
N_CORES = 8
CH = 512          # matmul moving free dim (score tile columns)
SUPER = 4         # pass-2 n-chunks per DMA super-chunk (2 MB bf16 DMAs)
SUPER1 = 8        # pass-1 n-chunks per DMA super-chunk (2 MB fp8 DMAs)
E_SCALE = 32.0    # fp8 range scaling for E (pass 1); scores come out x256
Q_SCALE = 8.0     # fp8 range scaling for q_hat (pass 1)
SCORE_SCALE = E_SCALE * Q_SCALE
FILTER_MARGIN = 0.02  # fp8 score uncertainty covered by the pass-2 trigger
K_OUT = 5
THRESH = 0.3
EPS = 1e-8
WEIGHTS = (0.4, 0.4, 0.2)

_cache = {}


def _build(ns, extract, split_waits=True):
    """Per-core Bass program for a shard of ns memory rows.

    extract=False: pass-1 flag program — survivor pre-filter only.
    extract=True:  pass-2 program — top-8 values+indices per 512-chunk.

    The DMA path is descriptor-count-bound (~128 descriptors per dma_start,
    one per partition, independent of size until the per-engine byte rate
    caps out at ~16 KB descriptors). So: the query lhsT is packed into the
    head of piece 0's per-partition data (no separate qt DMA slot), the
    shard streams as 5 big pieces alternating between the two HWDGE rings
    (sync + scalar) so both descriptor generators run, every piece stays
    resident in SBUF (no ring-reuse dependencies), and the pass-1 output is
    reduced on-device to a [1, 2] summary so the final DMA is one
    descriptor instead of 128."""
    import concourse.bass as bass
    import concourse.mybir as mybir
    from concourse.tile import TileContext
    from concourse.masks import make_identity
    from contextlib import ExitStack

    f32 = mybir.dt.float32
    bf16 = mybir.dt.bfloat16
    fp8 = mybir.dt.float8e4
    u16 = mybir.dt.uint16
    Act = mybir.ActivationFunctionType

    edt = bf16 if extract else fp8
    n_chunks = ns // CH
    qt_elems = 2 * 4 * 128          # per-partition lhsT elements
    ck_elems = 4 * CH               # per-partition elements per 512-chunk
    total_elems = qt_elems + n_chunks * ck_elems

    # chunk counts per piece and issuing ring (A=sync, B=scalar)
    piece_chunks = [4, 8, 8, 8, 4]
    assert sum(piece_chunks) == n_chunks

    nc = bass.Bass(trn_type="TRN2")

    et_d = nc.dram_tensor("et", [128, total_elems], edt,
                          kind="ExternalInput")
    if extract:
        vals_d = nc.dram_tensor("vals", [B, n_chunks * 8], f32,
                                kind="ExternalOutput")
        idx_d = nc.dram_tensor("idx", [B, n_chunks * 8], u16,
                               kind="ExternalOutput")
        vals_ap = vals_d.ap()
        idx_ap = idx_d.ap()
    else:
        flags_d = nc.dram_tensor("flags", [2, 128], f32,
                                 kind="ExternalOutput")
        flags_ap = flags_d.ap()

    et_ap = et_d.ap()

    with TileContext(nc) as tc, ExitStack() as ctx:
        consts = ctx.enter_context(tc.tile_pool(name="consts", bufs=1))
        spool = ctx.enter_context(tc.tile_pool(name="spool", bufs=2))
        psum_s = ctx.enter_context(tc.tile_pool(name="psum_s", bufs=7,
                                                space="PSUM"))
        psum_f = ctx.enter_context(tc.tile_pool(name="psum_f", bufs=1,
                                                space="PSUM"))

        if not extract:
            nthr = consts.tile([128, 1], f32, name="nthr")
            nc.vector.memset(nthr, float(-(THRESH - FILTER_MARGIN)))
            identity = consts.tile([128, 128], f32, name="identity")
            make_identity(nc, identity)

        # Stream the shard: piece 0 carries the query lhsT in its head.
        rings = [nc.sync, nc.scalar]
        pieces = []
        off = 0
        for i, pch in enumerate(piece_chunks):
            elems = pch * ck_elems + (qt_elems if i == 0 else 0)
            pt = consts.tile([128, elems], edt, name=f"piece{i}")
            rings[i % 2].dma_start(pt, et_ap[:, off:off + elems])
            off += elems
            pieces.append(pt)

        qT = pieces[0][:, :qt_elems].rearrange(
            "p (h k m) -> p h k m", h=2, k=4)
        chunk_views = []
        for i, pt in enumerate(pieces):
            base = qt_elems if i == 0 else 0
            v = pt[:, base:].rearrange(
                "p (c k n) -> p c k n", c=piece_chunks[i], k=4)
            for cc in range(piece_chunks[i]):
                chunk_views.append((v, cc))

        if extract:
            cv = [consts.tile([128, n_chunks * 8], f32, name=f"cv{h}")
                  for h in range(2)]
            ci = [consts.tile([128, n_chunks * 8], u16, name=f"ci{h}")
                  for h in range(2)]
        else:
            flags_dve = consts.tile([128, 40], f32, name="flags_dve")
            flags_act = consts.tile([128, 24], f32, name="flags_act")
            i_dve = i_act = 0

        for c in range(n_chunks):
            ev, ecs = chunk_views[c]
            for half in range(2):
                ps = psum_s.tile([128, CH], f32, tag="S")
                if extract:
                    for kb in range(4):
                        nc.tensor.matmul(
                            ps, qT[:, half, kb, :], ev[:, ecs, kb, :],
                            start=(kb == 0), stop=(kb == 3),
                        )
                    nc.vector.max(
                        out=cv[half][:, c * 8:(c + 1) * 8], in_=ps)
                    nc.vector.max_index(
                        out=ci[half][:, c * 8:(c + 1) * 8],
                        in_max=cv[half][:, c * 8:(c + 1) * 8],
                        in_values=ps)
                else:
                    for j in range(2):
                        nc.tensor.matmul(
                            ps, qT[:, half, 2 * j:2 * j + 2, :],
                            ev[:, ecs, 2 * j:2 * j + 2, :],
                            start=(j == 0), stop=(j == 1),
                            perf_mode=mybir.MatmulPerfMode.DoubleRow,
                        )
                    if half == 0 or c % 4 == 3:
                        # DVE (40 of 64 tiles): raw per-row max of the
                        # (x SCORE_SCALE) score tile.
                        nc.vector.tensor_reduce(
                            flags_dve[:, i_dve:i_dve + 1], ps,
                            axis=mybir.AxisListType.X,
                            op=mybir.AluOpType.max)
                        i_dve += 1
                    else:
                        # ACT (24 of 64): sum of relu(S - thresh) > 0
                        # iff any survivor (scale folds out the x256).
                        rsc = spool.tile([128, CH], bf16, tag="rsc")
                        nc.scalar.activation(
                            rsc, ps, Act.Relu,
                            scale=float(1.0 / SCORE_SCALE),
                            bias=nthr,
                            accum_out=flags_act[:, i_act:i_act + 1])
                        i_act += 1

        if extract:
            for half in range(2):
                nc.sync.dma_start(
                    vals_ap[half * 128:(half + 1) * 128, :], cv[half])
                nc.sync.dma_start(
                    idx_ap[half * 128:(half + 1) * 128, :], ci[half])
        else:
            # Reduce 64 per-row flags to one (dve_max, act_max) pair per
            # row, then PE-transpose so the output DMA covers 2 partitions
            # (2 descriptors) instead of 128.
            fsum = consts.tile([128, 2], f32, name="fsum")
            nc.vector.tensor_reduce(
                fsum[:, 0:1], flags_dve, axis=mybir.AxisListType.X,
                op=mybir.AluOpType.max)
            nc.vector.tensor_reduce(
                fsum[:, 1:2], flags_act, axis=mybir.AxisListType.X,
                op=mybir.AluOpType.max)
            ft = psum_f.tile([2, 128], f32, name="ft")
            nc.tensor.transpose(ft, fsum, identity)
            fredT = consts.tile([2, 128], f32, name="fredT")
            nc.scalar.activation(fredT, ft, Act.Copy)
            nc.sync.dma_start(flags_ap, fredT)

    if split_waits:
        _split_tsp_waits(nc, mybir)
    return nc


def _split_tsp_waits(nc, mybir):
    """This walrus build rejects ANY instruction carrying more than one
    sync-wait command in its encoding (TensorScalarPtr at birverifier;
    LdWeights/Matmult/DMACopy at codegen's setupSyncWait — verified
    empirically: trimming every instruction to one wait compiles). Hoist
    excess waits onto same-engine NoOps inserted just before — engines
    execute their stream in order, so gating the NoOp gates the op. The
    emitted stream order is a valid topological order of Tile's dependency
    graph, so blocking the issuing sequencer on a hoisted wait cannot
    deadlock."""
    skip = {"NoOp"}
    fn = nc.m.functions[0]
    for blk in fn.blocks:
        insts = list(blk.instructions)
        new_insts = []
        changed = False
        for ins in insts:
            si = ins.sync_info
            waits = list(si.on_wait) if si is not None and si.on_wait else []
            if ins.opcode not in skip and len(waits) > 1:
                for wi, w in enumerate(waits[:-1]):
                    new_insts.append(mybir.InstNoOp(
                        name=f"{ins.name}-wn{wi}",
                        engine=ins.engine,
                        sync_info=mybir.SyncInfo(on_wait=[w], on_update=[]),
                    ))
                ins.sync_info = mybir.SyncInfo(
                    on_wait=waits[-1:],
                    on_update=list(si.on_update) if si.on_update else [],
                )
                changed = True
            new_insts.append(ins)
        if changed:
            blk.instructions = new_insts


def _get_program(ns, extract):
    key = (ns, extract)
    if key not in _cache:
        _cache[key] = _build(ns, extract)
    return _cache[key]


def build_index(query, mem_questions, mem_responses, mem_traces,
                mem_strengths):
    """Host-side index build: fold per-row normalization, bank weights and
    strengths into one combined matrix E (f32)."""
    q = np.ascontiguousarray(np.asarray(query, dtype=np.float32))
    s = np.asarray(mem_strengths, dtype=np.float32)
    E = None
    for w, M in zip(WEIGHTS,
                    (mem_questions, mem_responses, mem_traces)):
        M = np.asarray(M, dtype=np.float32)
        nrm = np.sqrt(np.einsum("nd,nd->n", M, M))
        a = (w * s / (nrm + EPS)).astype(np.float32)
        E = M * a[:, None] if E is None else E + M * a[:, None]
    return q, E


def pack_in_maps(q, E, extract):
    """Shard E along N and pre-pack each core's input stream: per-partition
    layout = [query lhsT | chunk 0 | chunk 1 | ...] in matmul (transposed)
    order; fp8 (x E_SCALE / x Q_SCALE) for the pass-1 filter, bf16 for
    pass-2. qt[p, half, kb, m] = q_hat[half*128 + m, kb*128 + p]; chunk
    block [p, c, kb, n'] = E[c*CH + n', kb*128 + p]."""
    import ml_dtypes

    qhat = q / (np.sqrt(np.einsum("bd,bd->b", q, q))[:, None] + EPS)
    if extract:
        Eq = E.astype(ml_dtypes.bfloat16)
        qt = qhat.astype(ml_dtypes.bfloat16)
    else:
        Eq = (E * E_SCALE).astype(ml_dtypes.float8_e4m3)
        qt = (qhat * Q_SCALE).astype(ml_dtypes.float8_e4m3)
    qt = np.ascontiguousarray(
        qt.reshape(2, 128, 4, 128).transpose(3, 0, 2, 1)).reshape(128, -1)

    n = Eq.shape[0]
    ns = n // N_CORES
    n_chunks = ns // CH
    in_maps = []
    for c in range(N_CORES):
        Ec = Eq[c * ns:(c + 1) * ns]
        pk = Ec.reshape(n_chunks, CH, 4, 128).transpose(3, 0, 2, 1)
        pk = pk.reshape(128, n_chunks * 4 * CH)
        arr = np.ascontiguousarray(np.concatenate([qt, pk], axis=1))
        in_maps.append({"et": arr})
    return in_maps, ns


def fill_output(nrows, k):
    """All-rows-empty output: value -1.0, smallest indices (the reference's
    top_k tie-break over the uniform -1.0 masked array)."""
    vals = np.full((nrows, k), -1.0, dtype=np.float32)
    idx = np.tile(np.arange(k, dtype=np.int32), (nrows, 1))
    return vals, idx


def merge_candidates(per_core, ns, k):
    """Gather n_chunks x 8 raw-score candidates per core per row (indices
    chunk-local), apply the 0.3 threshold mask, and reduce to the global
    top-k (value desc, global index asc) — matching jax.lax.top_k on the
    masked array.

    Exactness of the -1 fills: a fill slot only occurs when fewer than k
    values globally exceed the threshold, in which case every survivor is
    within its chunk's top-8, so the survivor set is complete; the -1
    entries of the reference's top-k are then the smallest global indices
    not occupied by survivors (all masked entries tie at -1; top_k breaks
    ties by the lowest index)."""
    n_chunks = ns // CH
    coff = np.repeat(np.arange(n_chunks) * CH, 8)[None, :]
    cand_vals = np.concatenate(
        [np.asarray(r["vals"], dtype=np.float32) for r in per_core], axis=1)
    cand_idx = np.concatenate(
        [r["idx"].astype(np.int64) + coff + c * ns
         for c, r in enumerate(per_core)],
        axis=1,
    )
    masked_vals = np.where(cand_vals > THRESH, cand_vals, -np.inf)
    order1 = np.argsort(cand_idx, axis=1, kind="stable")
    v1 = np.take_along_axis(masked_vals, order1, axis=1)
    i1 = np.take_along_axis(cand_idx, order1, axis=1)
    order2 = np.argsort(-v1, axis=1, kind="stable")
    vals = np.take_along_axis(v1, order2, axis=1)[:, :k].copy()
    idx = np.take_along_axis(i1, order2, axis=1)[:, :k].copy()
    # Fill non-survivor slots with (-1.0, smallest free global indices).
    nrows = vals.shape[0]
    for r in range(nrows):
        m = int((vals[r] > -np.inf).sum())
        if m >= k:
            continue
        taken = set(int(x) for x in idx[r, :m])
        fill = []
        cand = 0
        while len(fill) < k - m:
            if cand not in taken:
                fill.append(cand)
            cand += 1
        vals[r, m:] = -1.0
        idx[r, m:] = fill
    return vals.astype(np.float32), idx.astype(np.int32)


def _install_ntff_shim():
    """Register the axon NTFF profile hook (the agent image lacks
    antenv.axon_hooks; recreate it per the documented ctypes C ABI)."""
    import sys as _sys
    import types
    import ctypes
    import contextlib

    if "antenv.axon_hooks" in _sys.modules:
        return
    so_path = "/opt/axon/libaxon_pjrt.so"
    lib = ctypes.CDLL(so_path)
    if not hasattr(lib, "axon_start_nrt_profile"):
        return
    lib.axon_start_nrt_profile.argtypes = [
        ctypes.POINTER(ctypes.c_int64), ctypes.c_size_t]
    lib.axon_start_nrt_profile.restype = ctypes.c_int64
    lib.axon_stop_nrt_profile.argtypes = [ctypes.c_char_p]
    lib.axon_stop_nrt_profile.restype = ctypes.c_int64

    @contextlib.contextmanager
    def _hook(output_dir, device_ids):
        import jax
        jax.devices()
        if device_ids:
            ids = (ctypes.c_int64 * len(device_ids))(*device_ids)
            rc = lib.axon_start_nrt_profile(ids, len(device_ids))
        else:
            rc = lib.axon_start_nrt_profile(None, 0)
        if rc != 0:
            raise RuntimeError(f"axon_start_nrt_profile rc={rc}")
        try:
            yield
        finally:
            n = lib.axon_stop_nrt_profile(str(output_dir).encode())
            print(f"ntff profile: {n} file(s) written to {output_dir}",
                  file=_sys.stderr)

    mod = types.ModuleType("antenv.axon_hooks")
    mod._hook = _hook
    mod.get_axon_ntff_profile_hook = lambda: _hook
    mod.set_axon_ntff_profile_hook = lambda h: None
    _sys.modules["antenv.axon_hooks"] = mod


def kernel(query, mem_questions, mem_responses, mem_traces, mem_strengths,
           top_k, _trace=False, _results_box=None, _force_extract=False):
    from concourse import bass_utils

    if _trace:
        _install_ntff_shim()

    k = int(top_k)
    assert k <= 8
    q, E = build_index(
        query, mem_questions, mem_responses, mem_traces, mem_strengths)

    # Pass 1: per-(row, chunk) survivor flags — the threshold pre-filter.
    in_maps1, ns = pack_in_maps(q, E, extract=False)
    nc1 = _get_program(ns, extract=False)
    res1 = bass_utils.run_bass_kernel_spmd(
        nc1, in_maps1, core_ids=list(range(N_CORES)), trace=_trace)
    if _results_box is not None:
        _results_box.append(res1)

    def _has_survivor(r):
        f = np.asarray(r["flags"], dtype=np.float32)
        # Row 0: per-row max raw score (x SCORE_SCALE) over DVE-flagged
        # tiles; row 1: per-row max relu-sum (> 0 iff any survivor).
        return bool(f[0].max() > (THRESH - FILTER_MARGIN) * SCORE_SCALE
                    or f[1].max() > 0.0)

    any_survivor = any(_has_survivor(r) for r in res1.results)

    if not (any_survivor or _force_extract):
        return fill_output(B, k)

    # Pass 2: some candidate beats the threshold — run full top-8
    # extraction (bf16) and merge exactly.
    in_maps2, ns = pack_in_maps(q, E, extract=True)
    nc2 = _get_program(ns, extract=True)
    res2 = bass_utils.run_bass_kernel_spmd(
        nc2, in_maps2, core_ids=list(range(N_CORES)), trace=_trace)
    if _results_box is not None:
        _results_box.append(res2)
    return merge_candidates(res2.results, ns, k)


# revision 23
# speedup vs baseline: 1.0861x; 1.0861x over previous
"""Distributed kNN retrieval kernel for Trainium2 (8 NeuronCores).

Computes, for query batch B=256 against three memory banks of N=131072 rows
(D=512): combined = (0.4*cos(q,Mq) + 0.4*cos(q,Mr) + 0.2*cos(q,Mt)) * strength,
masked below 0.3 to -1.0, then top-5 values + indices per query row
(ties broken by the lowest index, matching jax.lax.top_k).

The memory-side math is query-independent: cos(q, M_b) = q_hat . M_b_hat, so
  combined = q_hat @ E^T   with   E = sum_b w_b*strength/(||M_b||+eps) * M_b.
E is an index-time artifact (a real retrieval system stores normalized,
weighted embeddings); the host folds the three banks into E once, pre-packs
it in matmul (transposed) layout, and quantizes to bf16. The device does all
the query-dependent work.

The reference's 0.3 similarity threshold masks sub-threshold candidates to
-1.0, so the top-k output only ever contains above-threshold survivors (plus
deterministic -1/index fills). The kernel exploits this with the standard
threshold-pruned retrieval structure:

  Pass 1 (always): shard E along N across the 8 cores; each core normalizes
  the queries, streams its shard through the Tensor engine (q_hat @ E^T in
  [128, 512] PSUM tiles), and reduces each score tile to a per-(row, chunk)
  max on the DVE (tensor_reduce). The host gathers the tiny flag tensors and
  compares against the threshold.

  Pass 2 (only if some flag exceeds the threshold): rerun the shard with full
  top-8 extraction per 512-column chunk (DVE max/max_index, stable
  ascending-index tie-break), gather 8*32*8 candidates per row, and reduce to
  the global top-k on the host (value desc, index asc). Exactness: any
  element of the global top-5 has at most 4 elements above it anywhere, so it
  is inside its chunk's top-8.

  Rows with no survivors take the reference's tie-break on the -1.0 masked
  entries: value -1.0 with the smallest unoccupied indices, which the host
  emits directly.
"""

import sys

if "/opt/trn_rl_repo" not in sys.path:
    sys.path.insert(0, "/opt/trn_rl_repo")

import numpy as np

B = 256
D = 512
N_CORES = 8
CH = 512          # matmul moving free dim (score tile columns)
SUPER = 4         # pass-2 n-chunks per DMA super-chunk (2 MB bf16 DMAs)
SUPER1 = 8        # pass-1 n-chunks per DMA super-chunk (2 MB fp8 DMAs)
E_SCALE = 32.0    # fp8 range scaling for E (pass 1); scores come out x256
Q_SCALE = 8.0     # fp8 range scaling for q_hat (pass 1)
SCORE_SCALE = E_SCALE * Q_SCALE
FILTER_MARGIN = 0.02  # fp8 score uncertainty covered by the pass-2 trigger
K_OUT = 5
THRESH = 0.3
EPS = 1e-8
WEIGHTS = (0.4, 0.4, 0.2)

_cache = {}


def _build(ns, extract, split_waits=True):
    """Per-core Bass program for a shard of ns memory rows.

    extract=False: pass-1 flag program — survivor pre-filter only.
    extract=True:  pass-2 program — top-8 values+indices per 512-chunk.

    The DMA path is descriptor-count-bound (~128 descriptors per dma_start,
    one per partition, independent of size until the per-engine byte rate
    caps out at ~16 KB descriptors). So: the query lhsT is packed into the
    head of piece 0's per-partition data (no separate qt DMA slot), the
    shard streams as 5 big pieces alternating between the two HWDGE rings
    (sync + scalar) so both descriptor generators run, every piece stays
    resident in SBUF (no ring-reuse dependencies), and the pass-1 output is
    reduced on-device to a [1, 2] summary so the final DMA is one
    descriptor instead of 128."""
    import concourse.bass as bass
    import concourse.mybir as mybir
    from concourse.tile import TileContext
    from concourse.masks import make_identity
    from contextlib import ExitStack

    f32 = mybir.dt.float32
    bf16 = mybir.dt.bfloat16
    fp8 = mybir.dt.float8e4
    u16 = mybir.dt.uint16
    Act = mybir.ActivationFunctionType

    edt = bf16 if extract else fp8
    n_chunks = ns // CH
    qt_elems = 2 * 4 * 128          # per-partition lhsT elements
    ck_elems = 4 * CH               # per-partition elements per 512-chunk
    total_elems = qt_elems + n_chunks * ck_elems

    # (chunk count, ring) per piece; ~1 MB pieces alternating rings so
    # landings interleave in chunk order and neither ring falls behind.
    piece_plan = [(2, 0), (4, 1), (4, 0), (4, 1), (4, 0), (4, 1),
                  (4, 0), (4, 1), (2, 0)]
    piece_chunks = [p for p, _ in piece_plan]
    assert sum(piece_chunks) == n_chunks

    nc = bass.Bass(trn_type="TRN2")

    et_d = nc.dram_tensor("et", [128, total_elems], edt,
                          kind="ExternalInput")
    if extract:
        vals_d = nc.dram_tensor("vals", [B, n_chunks * 8], f32,
                                kind="ExternalOutput")
        idx_d = nc.dram_tensor("idx", [B, n_chunks * 8], u16,
                               kind="ExternalOutput")
        vals_ap = vals_d.ap()
        idx_ap = idx_d.ap()
    else:
        flags_d = nc.dram_tensor("flags", [2, 128], f32,
                                 kind="ExternalOutput")
        flags_ap = flags_d.ap()

    et_ap = et_d.ap()

    with TileContext(nc) as tc, ExitStack() as ctx:
        consts = ctx.enter_context(tc.tile_pool(name="consts", bufs=1))
        spool = ctx.enter_context(tc.tile_pool(name="spool", bufs=2))
        psum_s = ctx.enter_context(tc.tile_pool(name="psum_s", bufs=8,
                                                space="PSUM"))

        if not extract:
            nthr = consts.tile([128, 1], f32, name="nthr")
            nc.vector.memset(nthr, float(-(THRESH - FILTER_MARGIN)))
            identity = consts.tile([128, 128], f32, name="identity")
            make_identity(nc, identity)

        # Stream the shard: piece 0 carries the query lhsT in its head.
        rings = [nc.sync, nc.scalar]
        pieces = []
        off = 0
        for i, (pch, ring) in enumerate(piece_plan):
            elems = pch * ck_elems + (qt_elems if i == 0 else 0)
            pt = consts.tile([128, elems], edt, name=f"piece{i}")
            rings[ring].dma_start(pt, et_ap[:, off:off + elems])
            off += elems
            pieces.append(pt)

        qT = pieces[0][:, :qt_elems].rearrange(
            "p (h k m) -> p h k m", h=2, k=4)
        chunk_views = []
        for i, pt in enumerate(pieces):
            base = qt_elems if i == 0 else 0
            v = pt[:, base:].rearrange(
                "p (c k n) -> p c k n", c=piece_chunks[i], k=4)
            for cc in range(piece_chunks[i]):
                chunk_views.append((v, cc))

        if extract:
            cv = [consts.tile([128, n_chunks * 8], f32, name=f"cv{h}")
                  for h in range(2)]
            ci = [consts.tile([128, n_chunks * 8], u16, name=f"ci{h}")
                  for h in range(2)]
        else:
            flags_dve = consts.tile([128, 40], f32, name="flags_dve")
            flags_act = consts.tile([128, 24], f32, name="flags_act")
            i_dve = i_act = 0

        for c in range(n_chunks):
            ev, ecs = chunk_views[c]
            for half in range(2):
                ps = psum_s.tile([128, CH], f32, tag="S")
                if extract:
                    for kb in range(4):
                        nc.tensor.matmul(
                            ps, qT[:, half, kb, :], ev[:, ecs, kb, :],
                            start=(kb == 0), stop=(kb == 3),
                        )
                    nc.vector.max(
                        out=cv[half][:, c * 8:(c + 1) * 8], in_=ps)
                    nc.vector.max_index(
                        out=ci[half][:, c * 8:(c + 1) * 8],
                        in_max=cv[half][:, c * 8:(c + 1) * 8],
                        in_values=ps)
                else:
                    for j in range(2):
                        nc.tensor.matmul(
                            ps, qT[:, half, 2 * j:2 * j + 2, :],
                            ev[:, ecs, 2 * j:2 * j + 2, :],
                            start=(j == 0), stop=(j == 1),
                            perf_mode=mybir.MatmulPerfMode.DoubleRow,
                        )
                    if half == 0 or c % 4 == 3:
                        # DVE (40 of 64 tiles): raw per-row max of the
                        # (x SCORE_SCALE) score tile.
                        nc.vector.tensor_reduce(
                            flags_dve[:, i_dve:i_dve + 1], ps,
                            axis=mybir.AxisListType.X,
                            op=mybir.AluOpType.max)
                        i_dve += 1
                    else:
                        # ACT (24 of 64): sum of relu(S - thresh) > 0
                        # iff any survivor (scale folds out the x256).
                        rsc = spool.tile([128, CH], bf16, tag="rsc")
                        nc.scalar.activation(
                            rsc, ps, Act.Relu,
                            scale=float(1.0 / SCORE_SCALE),
                            bias=nthr,
                            accum_out=flags_act[:, i_act:i_act + 1])
                        i_act += 1

        if extract:
            for half in range(2):
                nc.sync.dma_start(
                    vals_ap[half * 128:(half + 1) * 128, :], cv[half])
                nc.sync.dma_start(
                    idx_ap[half * 128:(half + 1) * 128, :], ci[half])
        else:
            # Reduce 64 per-row flags to one (dve_max, act_max) pair per
            # row, then PE-transpose so the output DMA covers 2 partitions
            # (2 descriptors) instead of 128.
            fsum = consts.tile([128, 2], f32, name="fsum")
            nc.vector.tensor_reduce(
                fsum[:, 0:1], flags_dve, axis=mybir.AxisListType.X,
                op=mybir.AluOpType.max)
            nc.vector.tensor_reduce(
                fsum[:, 1:2], flags_act, axis=mybir.AxisListType.X,
                op=mybir.AluOpType.max)
            ftile = psum_s.tile([128, CH], f32, tag="S")
            ft = ftile[0:2, 0:128]
            nc.tensor.transpose(ft, fsum, identity)
            fredT = consts.tile([2, 128], f32, name="fredT")
            nc.scalar.activation(fredT, ft, Act.Copy)
            nc.sync.dma_start(flags_ap, fredT)

    if split_waits:
        _split_tsp_waits(nc, mybir)
    return nc


def _split_tsp_waits(nc, mybir):
    """This walrus build rejects ANY instruction carrying more than one
    sync-wait command in its encoding (TensorScalarPtr at birverifier;
    LdWeights/Matmult/DMACopy at codegen's setupSyncWait — verified
    empirically: trimming every instruction to one wait compiles). Hoist
    excess waits onto same-engine NoOps inserted just before — engines
    execute their stream in order, so gating the NoOp gates the op. The
    emitted stream order is a valid topological order of Tile's dependency
    graph, so blocking the issuing sequencer on a hoisted wait cannot
    deadlock."""
    skip = {"NoOp"}
    fn = nc.m.functions[0]
    for blk in fn.blocks:
        insts = list(blk.instructions)
        new_insts = []
        changed = False
        for ins in insts:
            si = ins.sync_info
            waits = list(si.on_wait) if si is not None and si.on_wait else []
            if ins.opcode not in skip and len(waits) > 1:
                for wi, w in enumerate(waits[:-1]):
                    new_insts.append(mybir.InstNoOp(
                        name=f"{ins.name}-wn{wi}",
                        engine=ins.engine,
                        sync_info=mybir.SyncInfo(on_wait=[w], on_update=[]),
                    ))
                ins.sync_info = mybir.SyncInfo(
                    on_wait=waits[-1:],
                    on_update=list(si.on_update) if si.on_update else [],
                )
                changed = True
            new_insts.append(ins)
        if changed:
            blk.instructions = new_insts


def _get_program(ns, extract):
    key = (ns, extract)
    if key not in _cache:
        _cache[key] = _build(ns, extract)
    return _cache[key]


def build_index(query, mem_questions, mem_responses, mem_traces,
                mem_strengths):
    """Host-side index build: fold per-row normalization, bank weights and
    strengths into one combined matrix E (f32)."""
    q = np.ascontiguousarray(np.asarray(query, dtype=np.float32))
    s = np.asarray(mem_strengths, dtype=np.float32)
    E = None
    for w, M in zip(WEIGHTS,
                    (mem_questions, mem_responses, mem_traces)):
        M = np.asarray(M, dtype=np.float32)
        nrm = np.sqrt(np.einsum("nd,nd->n", M, M))
        a = (w * s / (nrm + EPS)).astype(np.float32)
        E = M * a[:, None] if E is None else E + M * a[:, None]
    return q, E


def pack_in_maps(q, E, extract):
    """Shard E along N and pre-pack each core's input stream: per-partition
    layout = [query lhsT | chunk 0 | chunk 1 | ...] in matmul (transposed)
    order; fp8 (x E_SCALE / x Q_SCALE) for the pass-1 filter, bf16 for
    pass-2. qt[p, half, kb, m] = q_hat[half*128 + m, kb*128 + p]; chunk
    block [p, c, kb, n'] = E[c*CH + n', kb*128 + p]."""
    import ml_dtypes

    qhat = q / (np.sqrt(np.einsum("bd,bd->b", q, q))[:, None] + EPS)
    if extract:
        Eq = E.astype(ml_dtypes.bfloat16)
        qt = qhat.astype(ml_dtypes.bfloat16)
    else:
        Eq = (E * E_SCALE).astype(ml_dtypes.float8_e4m3)
        qt = (qhat * Q_SCALE).astype(ml_dtypes.float8_e4m3)
    qt = np.ascontiguousarray(
        qt.reshape(2, 128, 4, 128).transpose(3, 0, 2, 1)).reshape(128, -1)

    n = Eq.shape[0]
    ns = n // N_CORES
    n_chunks = ns // CH
    in_maps = []
    for c in range(N_CORES):
        Ec = Eq[c * ns:(c + 1) * ns]
        pk = Ec.reshape(n_chunks, CH, 4, 128).transpose(3, 0, 2, 1)
        pk = pk.reshape(128, n_chunks * 4 * CH)
        arr = np.ascontiguousarray(np.concatenate([qt, pk], axis=1))
        in_maps.append({"et": arr})
    return in_maps, ns


def fill_output(nrows, k):
    """All-rows-empty output: value -1.0, smallest indices (the reference's
    top_k tie-break over the uniform -1.0 masked array)."""
    vals = np.full((nrows, k), -1.0, dtype=np.float32)
    idx = np.tile(np.arange(k, dtype=np.int32), (nrows, 1))
    return vals, idx


def merge_candidates(per_core, ns, k):
    """Gather n_chunks x 8 raw-score candidates per core per row (indices
    chunk-local), apply the 0.3 threshold mask, and reduce to the global
    top-k (value desc, global index asc) — matching jax.lax.top_k on the
    masked array.

    Exactness of the -1 fills: a fill slot only occurs when fewer than k
    values globally exceed the threshold, in which case every survivor is
    within its chunk's top-8, so the survivor set is complete; the -1
    entries of the reference's top-k are then the smallest global indices
    not occupied by survivors (all masked entries tie at -1; top_k breaks
    ties by the lowest index)."""
    n_chunks = ns // CH
    coff = np.repeat(np.arange(n_chunks) * CH, 8)[None, :]
    cand_vals = np.concatenate(
        [np.asarray(r["vals"], dtype=np.float32) for r in per_core], axis=1)
    cand_idx = np.concatenate(
        [r["idx"].astype(np.int64) + coff + c * ns
         for c, r in enumerate(per_core)],
        axis=1,
    )
    masked_vals = np.where(cand_vals > THRESH, cand_vals, -np.inf)
    order1 = np.argsort(cand_idx, axis=1, kind="stable")
    v1 = np.take_along_axis(masked_vals, order1, axis=1)
    i1 = np.take_along_axis(cand_idx, order1, axis=1)
    order2 = np.argsort(-v1, axis=1, kind="stable")
    vals = np.take_along_axis(v1, order2, axis=1)[:, :k].copy()
    idx = np.take_along_axis(i1, order2, axis=1)[:, :k].copy()
    # Fill non-survivor slots with (-1.0, smallest free global indices).
    nrows = vals.shape[0]
    for r in range(nrows):
        m = int((vals[r] > -np.inf).sum())
        if m >= k:
            continue
        taken = set(int(x) for x in idx[r, :m])
        fill = []
        cand = 0
        while len(fill) < k - m:
            if cand not in taken:
                fill.append(cand)
            cand += 1
        vals[r, m:] = -1.0
        idx[r, m:] = fill
    return vals.astype(np.float32), idx.astype(np.int32)


def _install_ntff_shim():
    """Register the axon NTFF profile hook (the agent image lacks
    antenv.axon_hooks; recreate it per the documented ctypes C ABI)."""
    import sys as _sys
    import types
    import ctypes
    import contextlib

    if "antenv.axon_hooks" in _sys.modules:
        return
    so_path = "/opt/axon/libaxon_pjrt.so"
    lib = ctypes.CDLL(so_path)
    if not hasattr(lib, "axon_start_nrt_profile"):
        return
    lib.axon_start_nrt_profile.argtypes = [
        ctypes.POINTER(ctypes.c_int64), ctypes.c_size_t]
    lib.axon_start_nrt_profile.restype = ctypes.c_int64
    lib.axon_stop_nrt_profile.argtypes = [ctypes.c_char_p]
    lib.axon_stop_nrt_profile.restype = ctypes.c_int64

    @contextlib.contextmanager
    def _hook(output_dir, device_ids):
        import jax
        jax.devices()
        if device_ids:
            ids = (ctypes.c_int64 * len(device_ids))(*device_ids)
            rc = lib.axon_start_nrt_profile(ids, len(device_ids))
        else:
            rc = lib.axon_start_nrt_profile(None, 0)
        if rc != 0:
            raise RuntimeError(f"axon_start_nrt_profile rc={rc}")
        try:
            yield
        finally:
            n = lib.axon_stop_nrt_profile(str(output_dir).encode())
            print(f"ntff profile: {n} file(s) written to {output_dir}",
                  file=_sys.stderr)

    mod = types.ModuleType("antenv.axon_hooks")
    mod._hook = _hook
    mod.get_axon_ntff_profile_hook = lambda: _hook
    mod.set_axon_ntff_profile_hook = lambda h: None
    _sys.modules["antenv.axon_hooks"] = mod


def kernel(query, mem_questions, mem_responses, mem_traces, mem_strengths,
           top_k, _trace=False, _results_box=None, _force_extract=False):
    from concourse import bass_utils

    if _trace:
        _install_ntff_shim()

    k = int(top_k)
    assert k <= 8
    q, E = build_index(
        query, mem_questions, mem_responses, mem_traces, mem_strengths)

    # Pass 1: per-(row, chunk) survivor flags — the threshold pre-filter.
    in_maps1, ns = pack_in_maps(q, E, extract=False)
    nc1 = _get_program(ns, extract=False)
    res1 = bass_utils.run_bass_kernel_spmd(
        nc1, in_maps1, core_ids=list(range(N_CORES)), trace=_trace)
    if _results_box is not None:
        _results_box.append(res1)

    def _has_survivor(r):
        f = np.asarray(r["flags"], dtype=np.float32)
        # Row 0: per-row max raw score (x SCORE_SCALE) over DVE-flagged
        # tiles; row 1: per-row max relu-sum (> 0 iff any survivor).
        return bool(f[0].max() > (THRESH - FILTER_MARGIN) * SCORE_SCALE
                    or f[1].max() > 0.0)

    any_survivor = any(_has_survivor(r) for r in res1.results)

    if not (any_survivor or _force_extract):
        return fill_output(B, k)

    # Pass 2: some candidate beats the threshold — run full top-8
    # extraction (bf16) and merge exactly.
    in_maps2, ns = pack_in_maps(q, E, extract=True)
    nc2 = _get_program(ns, extract=True)
    res2 = bass_utils.run_bass_kernel_spmd(
        nc2, in_maps2, core_ids=list(range(N_CORES)), trace=_trace)
    if _results_box is not None:
        _results_box.append(res2)
    return merge_candidates(res2.results, ns, k)
